# revision 1
# baseline (speedup 1.0000x reference)
"""Trainium2 Bass kernel for ConformalGQA (b=4, t=1024, d=2048, 32 Q heads /
8 KV heads, hd=64, RoPE, causal, scores = (q.k - |q|^2/2 - |k|^2/2)/sqrt(hd)).

Sharding: 8-way tensor-parallel over heads. Core c owns Q heads 4c..4c+3 and
KV head c (Wq/Wk/Wv column-sharded, Wo row-sharded). Each core emits a full
(4096, 2048) partial output; the host sums the 8 partials.

Key device-side structure (per core, per batch):
 - x is transposed on the HOST once (xT [2048, 4096]); all matmuls contract
   over the partition dim so xT is the only layout ever needed.
 - QT/KT/VT projections with weights stationary -> head-transposed layouts.
 - Scores are computed as S^T (k on partitions, q on free dim):
     lhsT = Khat[:, kchunk] (65 x 128), rhs = Qhat (65 x q)
   where row 64 of Khat is ones and row 64 of Qhat is -0.5*|q|^2, so the
   q-norm term rides the contraction. The k-norm term is folded into the
   per-partition bias of the Exp activation (bias = -0.0625*|k|^2).
   Scores are <= 0 (conformal), so exp needs no max-subtraction pass.
 - P^T = exp(S^T/8 + bias) feeds attn@V directly (no transpose of P):
     yhat[65, q] += Vhat[kc].T @ P^T_kc,  Vhat = [V | 1] so row 64 of yhat
   accumulates the softmax denominator for free.
 - normalize via reciprocal + K=1 broadcast matmul, then Wo row-shard matmul.
All matmuls run in float32r (single-pass fp32 PE mode).
"""

import sys

for _p in ("/opt/trn_rl_repo",):
    if _p not in sys.path:
        sys.path.insert(0, _p)

import numpy as np
from contextlib import ExitStack

import concourse.bass as bass
import concourse.mybir as mybir
import concourse.tile as tile
from concourse import bacc
from concourse.bass_utils import run_bass_kernel_spmd

F32R = mybir.dt.float32r
F32 = mybir.dt.float32
AF = mybir.ActivationFunctionType

B, T, D, KV = 4, 1024, 2048, 512
H, HKV, HD = 32, 8, 64
P = 128
NCORES = 8
HPC = H // NCORES          # 4 q heads per core
DOUT = HPC * HD            # 256 q-proj cols per core
NDC = D // P               # 16 contraction chunks
NTC = T // P               # 8 token chunks per batch
ROPE_BASE = 10000.0

_COMPILED = {}


def _build_nc():
    nc = bacc.Bacc("TRN2", target_bir_lowering=False, debug=False,
                   num_devices=NCORES)

    xT = nc.dram_tensor("xT", [D, B * T], F32R, kind="ExternalInput")
    wq = nc.dram_tensor("wq", [P, NDC, DOUT], F32R, kind="ExternalInput")
    wkv = nc.dram_tensor("wkv", [P, NDC, 2 * HD], F32R, kind="ExternalInput")
    wo = nc.dram_tensor("wo", [P, 2, D], F32R, kind="ExternalInput")
    cc = nc.dram_tensor("cc", [P, T], F32, kind="ExternalInput")
    ss = nc.dram_tensor("ss", [P, T], F32, kind="ExternalInput")
    onesrow = nc.dram_tensor("onesrow", [1, T], F32R, kind="ExternalInput")
    ones64 = nc.dram_tensor("ones64", [64, 2], F32R, kind="ExternalInput")
    ones1x64 = nc.dram_tensor("ones1x64", [1, 64], F32R, kind="ExternalInput")
    ident = nc.dram_tensor("ident", [64, 64], F32R, kind="ExternalInput")
    triu = nc.dram_tensor("triu", [P, P], F32, kind="ExternalInput")
    onestc = nc.dram_tensor("onestc", [P, NTC], F32R, kind="ExternalInput")
    out = nc.dram_tensor("out", [B * T, D], F32, kind="ExternalOutput")

    with tile.TileContext(nc) as tc:
        with ExitStack() as ctx:
            cpool = ctx.enter_context(tc.tile_pool(name="consts", bufs=1))
            wpool = ctx.enter_context(tc.tile_pool(name="weights", bufs=1))
            xpool = ctx.enter_context(tc.tile_pool(name="x", bufs=4))
            qpool = ctx.enter_context(tc.tile_pool(name="qhat", bufs=4))
            kpool = ctx.enter_context(tc.tile_pool(name="khat", bufs=2))
            vpool = ctx.enter_context(tc.tile_pool(name="v", bufs=1))
            spool = ctx.enter_context(tc.tile_pool(name="scr", bufs=2))
            fpool = ctx.enter_context(tc.tile_pool(name="fr", bufs=3))
            ypool = ctx.enter_context(tc.tile_pool(name="ytn", bufs=2))
            opool = ctx.enter_context(tc.tile_pool(name="ostage", bufs=2))
            mpool = ctx.enter_context(tc.tile_pool(name="misc", bufs=2))
            ps = ctx.enter_context(tc.tile_pool(name="ps", bufs=4, space="PSUM"))

            # ---- constants / weights (loaded once) ----
            t_cc = cpool.tile([P, T], F32)
            nc.sync.dma_start(t_cc[:], cc.ap())
            t_ss = cpool.tile([P, T], F32)
            nc.sync.dma_start(t_ss[:], ss.ap())
            t_or = cpool.tile([1, T], F32R)
            nc.sync.dma_start(t_or[:], onesrow.ap())
            t_o64 = cpool.tile([64, 2], F32R)
            nc.sync.dma_start(t_o64[:], ones64.ap())
            t_o1x64 = cpool.tile([1, 64], F32R)
            nc.sync.dma_start(t_o1x64[:], ones1x64.ap())
            t_id = cpool.tile([64, 64], F32R)
            nc.sync.dma_start(t_id[:], ident.ap())
            t_tri = cpool.tile([P, P], F32)
            nc.sync.dma_start(t_tri[:], triu.ap())
            t_otc = cpool.tile([P, NTC], F32R)
            nc.sync.dma_start(t_otc[:], onestc.ap())

            t_wq = wpool.tile([P, NDC, DOUT], F32R)
            nc.sync.dma_start(t_wq[:], wq.ap())
            t_wkv = wpool.tile([P, NDC, 2 * HD], F32R)
            nc.sync.dma_start(t_wkv[:], wkv.ap())
            t_wo = wpool.tile([P, 2, D], F32R)
            nc.sync.dma_start(t_wo[:], wo.ap())

            xT3 = xT.ap().rearrange("(c p) t -> p c t", p=P)  # [128, 16, 4096]

            for b in range(B):
                tok0 = b * T

                # ---------- phase 1: load xT halves, project QT/KT/VT ------
                xh = []
                for qtr in range(4):
                    xt = xpool.tile([P, 4, T], F32R, tag="xh")
                    for i in range(4):
                        dc = qtr * 4 + i
                        nc.sync.dma_start(
                            xt[:, i, :], xT3[:, dc, tok0:tok0 + T])
                    xh.append(xt)

                def xsrc(dc):
                    return xh[dc // 4][:, dc % 4, :]

                qpss = [ps.tile([P, T], F32, tag="ps", name=f"qps_{b}_{i}")
                        for i in range(2)]
                kvps = ps.tile([P, T], F32, tag="ps")
                # stagger: qps0 completes first, then kvps, then qps1 --
                # rope/DVE work on early outputs overlaps the rest of the
                # projections on PE.
                for dc in range(NDC):
                    for hf in range(2):
                        nc.tensor.matmul(
                            qpss[0][:, hf * 512:(hf + 1) * 512],
                            t_wq[:, dc, 0:P],
                            xsrc(dc)[:, hf * 512:(hf + 1) * 512],
                            start=(dc == 0), stop=(dc == NDC - 1))
                for dc in range(NDC):
                    for hf in range(2):
                        nc.tensor.matmul(
                            kvps[:, hf * 512:(hf + 1) * 512], t_wkv[:, dc, :],
                            xsrc(dc)[:, hf * 512:(hf + 1) * 512],
                            start=(dc == 0), stop=(dc == NDC - 1))

                def rope_pair(qps, b=b):
                    t1 = spool.tile([P, T], F32, tag="t1")
                    nc.vector.tensor_mul(t1[:], qps[:], t_cc[:])
                    t2s = spool.tile([P, T], F32, tag="t2s")
                    for bp in (0, 64):
                        nc.vector.tensor_mul(
                            t2s[bp:bp + 32, :], qps[bp + 32:bp + 64, :],
                            t_ss[bp + 32:bp + 64, :])
                        nc.vector.tensor_mul(
                            t2s[bp + 32:bp + 64, :], qps[bp:bp + 32, :],
                            t_ss[bp:bp + 32, :])
                    out = []
                    for i in range(2):
                        qh = qpool.tile([65, T], F32R, tag="qhat")
                        bp = i * 64
                        nc.vector.tensor_add(
                            qh[0:64, :], t1[bp:bp + 64, :],
                            t2s[bp:bp + 64, :])
                        q2 = fpool.tile([64, T], F32R, tag="fr")
                        nc.vector.tensor_mul(
                            q2[:], qh[0:64, :].bitcast(F32),
                            qh[0:64, :].bitcast(F32))
                        nq = ps.tile([1, T], F32, tag="ps")
                        for hf in range(2):
                            nc.tensor.matmul(
                                nq[:, hf * 512:(hf + 1) * 512],
                                t_o64[:, 0:1],
                                q2[:, hf * 512:(hf + 1) * 512],
                                start=True, stop=True)
                        nc.scalar.mul(qh[64:65, :], nq[:], -0.5)
                        out.append(qh)
                    return out

                # rope pair 0 (overlaps qps1/kv matmuls below on PE)
                qhat = rope_pair(qpss[0])

                # KT -> khat (from kvps rows 0:64)
                khat = kpool.tile([65, T], F32R, tag="khat")
                nc.sync.dma_start(khat[64:65, :], onesrow.ap())
                t1 = spool.tile([P, T], F32, tag="t1")
                nc.vector.tensor_mul(t1[0:64, :], kvps[0:64, :], t_cc[0:64, :])
                t2s = spool.tile([P, T], F32, tag="t2s")
                nc.vector.tensor_mul(
                    t2s[0:32, :], kvps[32:64, :], t_ss[32:64, :])
                nc.vector.tensor_mul(
                    t2s[32:64, :], kvps[0:32, :], t_ss[0:32, :])
                nc.vector.tensor_add(
                    khat[0:64, :], t1[0:64, :], t2s[0:64, :])
                # knsq (transposed, per-partition bias): [128, 8]
                k2 = fpool.tile([64, T], F32R, tag="fr")
                nc.vector.tensor_mul(
                    k2[:], khat[0:64, :].bitcast(F32),
                    khat[0:64, :].bitcast(F32))
                nsqT = ps.tile([P, 2 * NTC], F32, tag="ps")
                for kc in range(NTC):
                    nc.tensor.matmul(
                        nsqT[:, 2 * kc:2 * kc + 2], k2[:, kc * P:(kc + 1) * P],
                        t_o64[:], start=True, stop=True)
                kbias = mpool.tile([P, NTC], F32, tag="kbias")
                nc.vector.tensor_scalar_mul(
                    kbias[:], nsqT[:].rearrange("p (c two) -> p c two", two=2)[:, :, 0],
                    -0.0625)

                # V (normal layout) with ones column, from kvps rows 64:128
                vt_sb = vpool.tile([64, T], F32R, tag="vt")
                nc.scalar.copy(vt_sb[:], kvps[64:128, :])
                vhat = vpool.tile([P, NTC, HD + 1], F32R, tag="vhat")
                nc.sync.dma_start(vhat[:, :, HD:HD + 1], onestc.ap())
                for tcn in range(NTC):
                    tp = ps.tile([P, 64], F32R, tag="ps")
                    nc.tensor.transpose(
                        tp[:], vt_sb[:, tcn * P:(tcn + 1) * P], t_id[:])
                    nc.scalar.copy(vhat[:, tcn, 0:HD], tp[:].bitcast(F32))

                # qps1 projections now (PE work overlapping the DVE above)
                for dc in range(NDC):
                    for hf in range(2):
                        nc.tensor.matmul(
                            qpss[1][:, hf * 512:(hf + 1) * 512],
                            t_wq[:, dc, P:2 * P],
                            xsrc(dc)[:, hf * 512:(hf + 1) * 512],
                            start=(dc == 0), stop=(dc == NDC - 1))
                qhat.extend(rope_pair(qpss[1]))

                # ---------- phase 2: attention per head ----------
                ytn = [ypool.tile([P, T], F32R, tag="ytn", name=f"ytn_{b}_{i}")
                       for i in range(2)]
                for h in range(HPC):
                    qh = qhat[h]
                    yhp = ps.tile([65, T], F32, tag="ps")
                    for kc in range(NTC):
                        q0 = kc * P
                        stp = ps.tile([P, T], F32, tag="ps")
                        c0 = q0
                        while c0 < T:
                            c1 = min(c0 + 512, T if c0 >= 512 else 512)
                            nc.tensor.matmul(
                                stp[:, c0:c1], khat[:, kc * P:(kc + 1) * P],
                                qh[:, c0:c1], start=True, stop=True)
                            c0 = c1
                        pt = fpool.tile([P, T], F32R, tag="fr")
                        nc.scalar.activation(
                            pt[:, q0:T], stp[:, q0:T], AF.Exp,
                            bias=kbias[:, kc:kc + 1], scale=0.125)
                        # causal mask on the diagonal block
                        nc.vector.tensor_mul(
                            pt[:, q0:q0 + P], pt[:, q0:q0 + P].bitcast(F32),
                            t_tri[:])
                        # attn@V accumulation over this k chunk
                        c0 = q0
                        while c0 < T:
                            c1 = min(c0 + 512, T if c0 >= 512 else 512)
                            nc.tensor.matmul(
                                yhp[:, c0:c1], vhat[:, kc, :], pt[:, c0:c1],
                                start=(kc == 0), stop=(kc == NTC - 1),
                                skip_group_check=True)
                            c0 = c1
                    # normalize: ytn rows = yhp[0:64] * bcast(1/denom)
                    rsb = mpool.tile([1, T], F32R, tag="rsb", bufs=1)
                    with nc.allow_low_precision(reason="f32r recip row"):
                        nc.vector.reciprocal(rsb[:], yhp[64:65, :])
                    dps = ps.tile([64, T], F32, tag="ps")
                    for hf in range(2):
                        nc.tensor.matmul(
                            dps[:, hf * 512:(hf + 1) * 512], t_o1x64[:],
                            rsb[:, hf * 512:(hf + 1) * 512],
                            start=True, stop=True)
                    ysb = spool.tile([64, T], F32, tag="t2")
                    nc.vector.tensor_copy(ysb[:], yhp[0:64, :])
                    bp = (h % 2) * 64
                    nc.vector.tensor_mul(
                        ytn[h // 2][bp:bp + 64, :], ysb[:], dps[:])

                # ---------- phase 3: output projection ----------
                for tcn in range(NTC):
                    for half in range(2):
                        ops_ = ps.tile([P, 1024], F32, tag="ps")
                        for hc in range(2):
                            lhsT = ytn[hc][:, tcn * P:(tcn + 1) * P]
                            for hf in range(2):
                                o0 = half * 1024 + hf * 512
                                nc.tensor.matmul(
                                    ops_[:, hf * 512:(hf + 1) * 512], lhsT,
                                    t_wo[:, hc, o0:o0 + 512],
                                    start=(hc == 0), stop=(hc == 1))
                        ostg = opool.tile([P, 1024], F32, tag="ostage")
                        nc.vector.tensor_copy(ostg[:], ops_[:])
                        nc.sync.dma_start(
                            out.ap()[tok0 + tcn * P: tok0 + (tcn + 1) * P,
                                     half * 1024:(half + 1) * 1024],
                            ostg[:])

    nc.finalize()
    return nc


def _host_consts():
    inv = 1.0 / (ROPE_BASE ** (np.arange(0, HD, 2, dtype=np.float32) / HD))
    ang = np.arange(T, dtype=np.float32)[:, None] * inv[None, :]  # [T, 32]
    cosr = np.cos(ang).T.astype(np.float32)                        # [32, T]
    sinr = np.sin(ang).T.astype(np.float32)
    cc = np.tile(cosr, (4, 1))                                     # [128, T]
    # signed sin table: +sin on x1 rows (j<32), -sin on x2 rows (j>=32);
    # reading row r of ssx multiplies the operand that LANDS shifted by +-32.
    ss = np.tile(np.concatenate([sinr, -sinr], axis=0), (2, 1))
    consts = {
        "cc": np.ascontiguousarray(cc),
        "ss": np.ascontiguousarray(ss),
        "onesrow": np.ones((1, T), np.float32),
        "ones64": np.ones((64, 2), np.float32),
        "ones1x64": np.ones((1, 64), np.float32),
        "ident": np.eye(64, dtype=np.float32),
        "triu": np.triu(np.ones((P, P), np.float32)),
        "onestc": np.ones((P, NTC), np.float32),
    }
    return consts


def kernel(x, Wq, Wk, Wv, Wo):
    x = np.asarray(x, np.float32)
    Wq = np.asarray(Wq, np.float32)
    Wk = np.asarray(Wk, np.float32)
    Wv = np.asarray(Wv, np.float32)
    Wo = np.asarray(Wo, np.float32)
    b, t, d = x.shape

    key = "nc"
    if key not in _COMPILED:
        _COMPILED[key] = _build_nc()
    nc = _COMPILED[key]

    xTh = np.ascontiguousarray(x.reshape(b * t, d).T)  # [2048, 4096]
    consts = _host_consts()

    in_maps = []
    for c in range(NCORES):
        wq_c = np.ascontiguousarray(
            Wq[:, c * DOUT:(c + 1) * DOUT].reshape(NDC, P, DOUT)
            .transpose(1, 0, 2))
        wkv_np = np.concatenate(
            [Wk[:, c * HD:(c + 1) * HD], Wv[:, c * HD:(c + 1) * HD]], axis=1)
        wkv_c = np.ascontiguousarray(
            wkv_np.reshape(NDC, P, 2 * HD).transpose(1, 0, 2))
        wo_c = np.ascontiguousarray(
            Wo[c * DOUT:(c + 1) * DOUT, :].reshape(2, P, d).transpose(1, 0, 2))
        m = {"xT": xTh, "wq": wq_c, "wkv": wkv_c, "wo": wo_c}
        m.update(consts)
        in_maps.append(m)

    res = run_bass_kernel_spmd(nc, in_maps, list(range(NCORES)))
    acc = res.results[0]["out"].astype(np.float32)
    for c in range(1, NCORES):
        acc = acc + res.results[c]["out"]
    return acc.reshape(b, t, d)


if __name__ == "__main__":
    rng = np.random.default_rng(0)
    x = rng.standard_normal((B, T, D), dtype=np.float32)
    Wq = (rng.standard_normal((D, D), dtype=np.float32) * 0.02)
    Wk = (rng.standard_normal((D, KV), dtype=np.float32) * 0.02)
    Wv = (rng.standard_normal((D, KV), dtype=np.float32) * 0.02)
    Wo = (rng.standard_normal((D, D), dtype=np.float32) * 0.02)
    y = kernel(x=x, Wq=Wq, Wk=Wk, Wv=Wv, Wo=Wo)
    print("out", y.shape, y.dtype, np.abs(y).max())



# revision 3
# speedup vs baseline: 1.8711x; 1.8711x over previous
"""Trainium2 Bass kernel for ConformalGQA, v2.

Math identical to reference modulo softmax shift invariance: the -0.5|q|^2
term in the scores is constant over the softmax (key) axis, so it is dropped
entirely. Scores become (q.k - 0.5|k|^2)/8, bounded above by |q|^2/16 ~ 6, so
exp never overflows fp32 and needs no max pass. The -0.5|k|^2/8 term rides
the per-partition bias of the Exp activation.

Sharding: 8-way tensor-parallel over heads (core c: Q heads 4c..4c+3, KV
head c). Each core emits a full (4096, 2048) bf16 partial; host sums.

Per core, per batch (t=1024):
 - xT chunks DMA'd bf16; Wq/Wk/Wv column shards projected with weights
   stationary into PSUM f32 chunks [128, 512].
 - RoPE: PSUM chunk evicted to SBUF f32 (Act), cos-mul + signed-sin
   shifted-muls (shift = +-32 partitions; muls on GPSIMD, add on DVE),
   emitted as bf16 qhat/khat. khat duplicated to partitions 64:128 so both
   heads of a pair run S-matmuls via tile_position (0,0)/(64,0).
 - S^T computed per (head, kc) into [128, <=512] PSUM chunks with k on
   partitions; chunk starts aligned down to 256 so fp32r/bf16 matmuls never
   hit the <256-free-dim 4x penalty; the over-computed region is zeroed by
   the causal mask (tri / [0|tri] tiles), fused across the head pair.
 - P^T = Exp(S^T/8 + bias) -> bf16, bias = -0.0625|k|^2.
 - PV: yhat[65, q] += [V|1].T @ P^T accumulated over kc in PSUM; row 64 is
   the softmax denominator.
 - normalize: reciprocal row, K=1 ones-matmul broadcast to 64 partitions,
   fused mul -> ytn bf16.
 - out proj: ytn as lhsT against Wo row-shard, PSUM chunks evicted bf16
   (alternating DVE/Act) and DMA'd out per 128-token row block.
"""

import sys

for _p in ("/opt/trn_rl_repo",):
    if _p not in sys.path:
        sys.path.insert(0, _p)

import numpy as np
import ml_dtypes
from contextlib import ExitStack

import concourse.bass as bass
import concourse.mybir as mybir
import concourse.tile as tile
from concourse import bacc
from concourse.bass_utils import run_bass_kernel_spmd

F32R = mybir.dt.float32r
F32 = mybir.dt.float32
BF16 = mybir.dt.bfloat16
AF = mybir.ActivationFunctionType
BF = ml_dtypes.bfloat16

B, T, D, KV = 4, 1024, 2048, 512
H, HKV, HD = 32, 8, 64
P = 128
NCORES = 8
HPC = H // NCORES          # 4 q heads per core
DOUT = HPC * HD            # 256 q-proj cols per core
NDC = D // P               # 16 contraction chunks
NTC = T // P               # 8 token chunks per batch
ROPE_BASE = 10000.0

_COMPILED = {}


def _chunks_for(kc):
    """Natural S/PV q-chunks for key block kc (bf16: any width is full
    rate). Chunks never straddle the 512 boundary (PSUM half split)."""
    q0 = kc * P
    out = []
    c0 = q0
    while c0 < T:
        c1 = min(T, 512 if c0 < 512 else T)
        out.append((c0, c1))
        c0 = c1
    return q0, out


def _build_nc():
    nc = bacc.Bacc("TRN2", target_bir_lowering=False, debug=False,
                   num_devices=NCORES)

    xT = nc.dram_tensor("xT", [D, B * T], BF16, kind="ExternalInput")
    wq = nc.dram_tensor("wq", [P, NDC, DOUT], BF16, kind="ExternalInput")
    wkv = nc.dram_tensor("wkv", [P, NDC, 2 * HD], BF16, kind="ExternalInput")
    wo = nc.dram_tensor("wo", [P, 2, D], BF16, kind="ExternalInput")
    cc = nc.dram_tensor("cc", [P, T], F32, kind="ExternalInput")
    ss = nc.dram_tensor("ss", [P, T], F32, kind="ExternalInput")
    tri2 = nc.dram_tensor("tri2", [P, 2, P], BF16, kind="ExternalInput")
    o64 = nc.dram_tensor("o64", [64, 2], F32R, kind="ExternalInput")
    o1x64 = nc.dram_tensor("o1x64", [1, 64], F32R, kind="ExternalInput")
    idb = nc.dram_tensor("idb", [64, 64], BF16, kind="ExternalInput")
    out = nc.dram_tensor("out", [B * T, D], BF16, kind="ExternalOutput")

    with tile.TileContext(nc) as tc:
        with ExitStack() as ctx:
            cpool = ctx.enter_context(tc.tile_pool(name="consts", bufs=1))
            wpool = ctx.enter_context(tc.tile_pool(name="weights", bufs=1))
            xpool = ctx.enter_context(tc.tile_pool(name="x", bufs=8))
            spool = ctx.enter_context(tc.tile_pool(name="stage", bufs=4))
            qpool = ctx.enter_context(tc.tile_pool(name="qk", bufs=2))
            vpool = ctx.enter_context(tc.tile_pool(name="v", bufs=2))
            fpool = ctx.enter_context(tc.tile_pool(name="pt", bufs=3))
            npool = ctx.enter_context(tc.tile_pool(name="norm", bufs=4))
            ypool = ctx.enter_context(tc.tile_pool(name="ytn", bufs=2))
            opool = ctx.enter_context(tc.tile_pool(name="ostg", bufs=3))
            psy = ctx.enter_context(tc.tile_pool(name="psy", bufs=2, space="PSUM"))
            pss = ctx.enter_context(tc.tile_pool(name="pss", bufs=4, space="PSUM"))
            psm = ctx.enter_context(tc.tile_pool(name="psm", bufs=2, space="PSUM"))

            # ---- early consts (needed by first projections/rope) ----
            t_wkv = wpool.tile([P, NDC, 2 * HD], BF16)
            nc.sync.dma_start(t_wkv[:], wkv.ap())
            t_wq = wpool.tile([P, NDC, DOUT], BF16)
            t_cc = cpool.tile([P, T], F32)
            t_ss = cpool.tile([P, T], F32)

            xT3 = xT.ap().rearrange("(c p) t -> p c t", p=P)  # [128, 16, 4096]

            def late_consts():
                t_tri2 = cpool.tile([P, 2, P], BF16)
                nc.sync.dma_start(t_tri2[:], tri2.ap())
                t_o64 = cpool.tile([64, 2], F32R)
                nc.sync.dma_start(t_o64[:], o64.ap())
                t_o1x64 = cpool.tile([1, 64], F32R)
                nc.sync.dma_start(t_o1x64[:], o1x64.ap())
                t_idb = cpool.tile([64, 64], BF16)
                nc.sync.dma_start(t_idb[:], idb.ap())
                t_wo = wpool.tile([P, 2, D], BF16)
                nc.sync.dma_start(t_wo[:], wo.ap())
                return t_tri2, t_o64, t_wo, t_o1x64, t_idb

            lc = None

            def rope_half(pj, dst, rows, half, sign_dup):
                """Evict PSUM proj chunk, rope it, write bf16 into dst."""
                c0 = half * 512
                sb = spool.tile([P, 512], F32, tag="qsb")
                nc.vector.tensor_copy(sb[0:rows, :], pj[0:rows, :])
                t1 = spool.tile([P, 512], F32, tag="t1")
                nc.vector.tensor_mul(
                    t1[0:rows, :], sb[0:rows, :], t_cc[0:rows, c0:c0 + 512])
                t2 = spool.tile([P, 512], F32, tag="t2")
                for bp2 in range(0, rows, 64):
                    nc.gpsimd.tensor_mul(
                        t2[bp2:bp2 + 32, :], sb[bp2 + 32:bp2 + 64, :],
                        t_ss[bp2 + 32:bp2 + 64, c0:c0 + 512])
                    nc.gpsimd.tensor_mul(
                        t2[bp2 + 32:bp2 + 64, :], sb[bp2:bp2 + 32, :],
                        t_ss[bp2:bp2 + 32, c0:c0 + 512])
                nc.vector.tensor_add(
                    dst[0:rows, c0:c0 + 512], t1[0:rows, :], t2[0:rows, :])
                if sign_dup:
                    nc.vector.tensor_copy(
                        dst[64:128, c0:c0 + 512], dst[0:64, c0:c0 + 512])

            def proj_rope_stage(b):
                """Load xT for batch b, project Q/K/V, rope, prep vh/kb."""
                nonlocal lc
                tok0 = b * T
                xts = []
                for qtr in range(4):
                    xt = xpool.tile([P, 4, T], BF16, tag="xt",
                                    name=f"xt_{b}_{qtr}")
                    if b == 0:
                        # fine-grained loads so batch-0 projections start
                        # as soon as the first contraction chunk lands
                        for i in range(4):
                            nc.sync.dma_start(
                                xt[:, i, :],
                                xT3[:, qtr * 4 + i, tok0:tok0 + T])
                        if qtr == 0:
                            nc.sync.dma_start(t_wq[:], wq.ap())
                        if qtr == 2:
                            nc.sync.dma_start(t_cc[:], cc.ap())
                            nc.sync.dma_start(t_ss[:], ss.ap())
                    else:
                        nc.sync.dma_start(
                            xt[:], xT3[:, qtr * 4:(qtr + 1) * 4, tok0:tok0 + T])
                    xts.append(xt)
                if b == 0:
                    lc = late_consts()

                def xsrc(dc):
                    return xts[dc // 4][:, dc % 4, :]

                qh = [qpool.tile([P, T], BF16, tag="qh", bufs=4,
                                 name=f"qh_{b}_{i}") for i in range(2)]
                kh = qpool.tile([P, T], BF16, tag="kh", name=f"kh_{b}")
                k2 = qpool.tile([64, T], F32R, tag="k2", name=f"k2_{b}")
                vt = vpool.tile([64, T], BF16, tag="vt", name=f"vt_{b}")

                # interleave kv and q-pair0 chunks so both pj slots
                # stream against arriving xT chunks; then q-pair1.
                def kv_chunk(half):
                    pj = psm.tile([P, 512], F32, tag="pj",
                                  name=f"kvpj_{b}_{half}")
                    for dc in range(NDC):
                        nc.tensor.matmul(
                            pj[:], t_wkv[:, dc, :],
                            xsrc(dc)[:, half * 512:(half + 1) * 512],
                            start=(dc == 0), stop=(dc == NDC - 1))
                    # kv proj out rows 0:64 = K dims, 64:128 = V dims.
                    nc.vector.tensor_copy(vt[:, half * 512:(half + 1) * 512],
                                          pj[64:128, :])
                    rope_half(pj, kh, 64, half, sign_dup=True)

                def q_chunk(pairi, half):
                    pj = psm.tile([P, 512], F32, tag="pj",
                                  name=f"qpj_{b}_{pairi}_{half}")
                    for dc in range(NDC):
                        nc.tensor.matmul(
                            pj[:],
                            t_wq[:, dc, pairi * P:(pairi + 1) * P],
                            xsrc(dc)[:, half * 512:(half + 1) * 512],
                            start=(dc == 0), stop=(dc == NDC - 1))
                    rope_half(pj, qh[pairi], 128, half, sign_dup=False)

                kv_chunk(0)
                q_chunk(0, 0)
                kv_chunk(1)
                q_chunk(0, 1)
                q_chunk(1, 0)
                q_chunk(1, 1)

                # |k|^2 -> per-partition bias  (transposed via PE)
                t_o64, t_idb = lc[1], lc[4]
                nc.scalar.activation(k2[:], kh[0:64, :], AF.Square)
                nsq = psm.tile([P, 512], F32, tag="pj", name=f"nsq_{b}")
                for kc in range(NTC):
                    nc.tensor.matmul(
                        nsq[:, 2 * kc:2 * kc + 2], k2[:, kc * P:(kc + 1) * P],
                        t_o64[:], start=True, stop=True)
                kb = qpool.tile([P, NTC], F32, tag="kb", name=f"kb_{b}")
                nc.vector.tensor_scalar_mul(
                    kb[:],
                    nsq[:, 0:2 * NTC]
                    .rearrange("p (c two) -> p c two", two=2)[:, :, 0],
                    -0.0625)

                # V transposed into [token, hd | 1] layout via PE transpose
                vh = vpool.tile([P, NTC, HD + 1], BF16, tag="vh",
                                name=f"vh_{b}")
                nc.vector.memset(vh[:, :, HD:HD + 1], 1.0)
                for tcn in range(NTC):
                    tp = pss.tile([P, 64], BF16, tag="stp", name=f"tp_{b}_{tcn}")
                    nc.tensor.transpose(
                        tp[:], vt[:, tcn * P:(tcn + 1) * P], t_idb[:])
                    nc.scalar.copy(vh[:, tcn, 0:HD], tp[:])
                return dict(b=b, qh=qh, kh=kh, kb=kb, vh=vh)

            def attn_out_stage(st):
                b, qh, kh, kb, vh = st["b"], st["qh"], st["kh"], st["kb"], st["vh"]
                tok0 = b * T
                t_tri2, t_o64, t_wo, t_o1x64, t_idb = lc
                ytn = [ypool.tile([P, T], BF16, tag="ytn", bufs=4,
                                  name=f"ytn_{b}_{i}") for i in range(2)]
                def normalize_half(yh_half, hq, pairi, bp, tag):
                    """One half of softmax-normalize as soon as its PV
                    contributions are complete; frees the yh slot early."""
                    rsb = npool.tile([1, 512], F32R, tag="rsb")
                    with nc.allow_low_precision(reason="recip row"):
                        nc.vector.reciprocal(rsb[:], yh_half[64:65, :])
                    ysb = npool.tile([64, 512], F32, tag="ysb")
                    nc.scalar.copy(ysb[:], yh_half[0:64, :])
                    dps = pss.tile([64, 512], F32, tag="stp",
                                   name=f"dps_{b}_{bp}_{hq}_{tag}")
                    nc.tensor.matmul(
                        dps[:], t_o1x64[:], rsb[:], start=True, stop=True)
                    nc.vector.tensor_mul(
                        ytn[pairi][bp:bp + 64, hq * 512:(hq + 1) * 512],
                        ysb[:], dps[:])

                for h in range(HPC):
                    pairi, bp = h // 2, 64 * (h % 2)
                    yhA = psy.tile([65, 512], F32, tag="yh",
                                   name=f"yhA_{b}_{h}")
                    yhB = psy.tile([65, 512], F32, tag="yh",
                                   name=f"yhB_{b}_{h}")
                    for kc in range(NTC):
                        q0 = kc * P
                        _, chs = _chunks_for(kc)
                        pt = fpool.tile([P, T], BF16, tag="pt", bufs=6)
                        for (c0, c1) in chs:
                            stp = pss.tile([P, 512], F32, tag="stp")
                            nc.tensor.matmul(
                                stp[:, 0:c1 - c0],
                                kh[bp:bp + 64, kc * P:(kc + 1) * P],
                                qh[pairi][bp:bp + 64, c0:c1],
                                start=True, stop=True,
                                tile_position=(bp, 0))
                            nc.scalar.activation(
                                pt[:, c0:c1], stp[:, 0:c1 - c0], AF.Exp,
                                bias=kb[:, kc:kc + 1], scale=0.125)
                        # causal mask on the diagonal block
                        meng = nc.vector if kc % 2 == 0 else nc.gpsimd
                        meng.tensor_mul(
                            pt[:, q0:q0 + P], pt[:, q0:q0 + P],
                            t_tri2[:, 0, :])
                        for (c0, c1) in chs:
                            half = yhA if c0 < 512 else yhB
                            off = 0 if c0 < 512 else 512
                            nc.tensor.matmul(
                                half[:, c0 - off:c1 - off], vh[:, kc, :],
                                pt[:, c0:c1],
                                start=(kc == 0),
                                stop=(kc == (3 if half is yhA else NTC - 1)),
                                skip_group_check=True)
                        if kc == 3:
                            normalize_half(yhA, 0, pairi, bp, "A")
                    normalize_half(yhB, 1, pairi, bp, "B")

                # ---------- output projection ----------
                for tcn in range(NTC):
                    ostg = opool.tile([P, D], BF16, tag="ostg")
                    for oc in range(4):
                        ops_ = pss.tile([P, 512], F32, tag="stp",
                                        name=f"ops_{b}_{tcn}_{oc}")
                        for hc in range(2):
                            nc.tensor.matmul(
                                ops_[:], ytn[hc][:, tcn * P:(tcn + 1) * P],
                                t_wo[:, hc, oc * 512:(oc + 1) * 512],
                                start=(hc == 0), stop=(hc == 1))
                        if oc % 2 == 1:
                            nc.scalar.copy(
                                ostg[:, oc * 512:(oc + 1) * 512], ops_[:])
                        else:
                            nc.vector.tensor_copy(
                                ostg[:, oc * 512:(oc + 1) * 512], ops_[:])
                    nc.sync.dma_start(
                        out.ap()[tok0 + tcn * P: tok0 + (tcn + 1) * P, :],
                        ostg[:])

            # software pipeline: proj/rope of b+1 issued (higher priority)
            # before attention/outproj of b so PE always has filler work.
            prev = proj_rope_stage(0)
            for b in range(1, B):
                cur = proj_rope_stage(b)
                attn_out_stage(prev)
                prev = cur
            attn_out_stage(prev)

    nc.finalize()
    return nc


def _host_consts():
    inv = 1.0 / (ROPE_BASE ** (np.arange(0, HD, 2, dtype=np.float32) / HD))
    ang = np.arange(T, dtype=np.float32)[:, None] * inv[None, :]  # [T, 32]
    cosr = np.cos(ang).T.astype(np.float32)                        # [32, T]
    sinr = np.sin(ang).T.astype(np.float32)
    cc = np.tile(cosr, (4, 1))                                     # [128, T]
    ss = np.tile(np.concatenate([sinr, -sinr], axis=0), (2, 1))
    tri = np.triu(np.ones((P, P), np.float32))
    tri2 = np.stack([tri, tri], axis=1)                            # [128,2,128]
    return {
        "cc": np.ascontiguousarray(cc),
        "ss": np.ascontiguousarray(ss),
        "tri2": np.ascontiguousarray(tri2.astype(BF)),
        "o64": np.ones((64, 2), np.float32),
        "o1x64": np.ones((1, 64), np.float32),
        "idb": np.eye(64, dtype=np.float32).astype(BF),
    }


def kernel(x, Wq, Wk, Wv, Wo):
    x = np.asarray(x, np.float32)
    Wq = np.asarray(Wq, np.float32)
    Wk = np.asarray(Wk, np.float32)
    Wv = np.asarray(Wv, np.float32)
    Wo = np.asarray(Wo, np.float32)
    b, t, d = x.shape

    key = "nc"
    if key not in _COMPILED:
        _COMPILED[key] = _build_nc()
    nc = _COMPILED[key]

    xTh = np.ascontiguousarray(x.reshape(b * t, d).T.astype(BF))  # [2048, 4096]
    consts = _host_consts()

    in_maps = []
    for c in range(NCORES):
        wq_c = np.ascontiguousarray(
            Wq[:, c * DOUT:(c + 1) * DOUT].reshape(NDC, P, DOUT)
            .transpose(1, 0, 2).astype(BF))
        wkv_np = np.concatenate(
            [Wk[:, c * HD:(c + 1) * HD], Wv[:, c * HD:(c + 1) * HD]], axis=1)
        wkv_c = np.ascontiguousarray(
            wkv_np.reshape(NDC, P, 2 * HD).transpose(1, 0, 2).astype(BF))
        wo_c = np.ascontiguousarray(
            Wo[c * DOUT:(c + 1) * DOUT, :].reshape(2, P, d)
            .transpose(1, 0, 2).astype(BF))
        m = {"xT": xTh, "wq": wq_c, "wkv": wkv_c, "wo": wo_c}
        m.update(consts)
        in_maps.append(m)

    res = run_bass_kernel_spmd(nc, in_maps, list(range(NCORES)))
    acc = res.results[0]["out"].astype(np.float32)
    for c in range(1, NCORES):
        acc = acc + res.results[c]["out"].astype(np.float32)
    return acc.reshape(b, t, d)


if __name__ == "__main__":
    rng = np.random.default_rng(0)
    x = rng.standard_normal((B, T, D), dtype=np.float32)
    Wq = (rng.standard_normal((D, D), dtype=np.float32) * 0.02)
    Wk = (rng.standard_normal((D, KV), dtype=np.float32) * 0.02)
    Wv = (rng.standard_normal((D, KV), dtype=np.float32) * 0.02)
    Wo = (rng.standard_normal((D, D), dtype=np.float32) * 0.02)
    y = kernel(x=x, Wq=Wq, Wk=Wk, Wv=Wv, Wo=Wo)
    print("out", y.shape, y.dtype, np.abs(y).max())


# revision 4
# speedup vs baseline: 2.1369x; 1.1421x over previous
"""Trainium2 Bass kernel for ConformalGQA, v2.

Math identical to reference modulo softmax shift invariance: the -0.5|q|^2
term in the scores is constant over the softmax (key) axis, so it is dropped
entirely. Scores become (q.k - 0.5|k|^2)/8, bounded above by |q|^2/16 ~ 6, so
exp never overflows fp32 and needs no max pass. The -0.5|k|^2/8 term rides
the per-partition bias of the Exp activation.

Sharding: 8-way tensor-parallel over heads (core c: Q heads 4c..4c+3, KV
head c). Each core emits a full (4096, 2048) bf16 partial; host sums.

Per core, per batch (t=1024):
 - xT chunks DMA'd bf16; Wq/Wk/Wv column shards projected with weights
   stationary into PSUM f32 chunks [128, 512].
 - RoPE: PSUM chunk evicted to SBUF f32 (Act), cos-mul + signed-sin
   shifted-muls (shift = +-32 partitions; muls on GPSIMD, add on DVE),
   emitted as bf16 qhat/khat. khat duplicated to partitions 64:128 so both
   heads of a pair run S-matmuls via tile_position (0,0)/(64,0).
 - S^T computed per (head, kc) into [128, <=512] PSUM chunks with k on
   partitions; chunk starts aligned down to 256 so fp32r/bf16 matmuls never
   hit the <256-free-dim 4x penalty; the over-computed region is zeroed by
   the causal mask (tri / [0|tri] tiles), fused across the head pair.
 - P^T = Exp(S^T/8 + bias) -> bf16, bias = -0.0625|k|^2.
 - PV: yhat[65, q] += [V|1].T @ P^T accumulated over kc in PSUM; row 64 is
   the softmax denominator.
 - normalize: reciprocal row, K=1 ones-matmul broadcast to 64 partitions,
   fused mul -> ytn bf16.
 - out proj: ytn as lhsT against Wo row-shard, PSUM chunks evicted bf16
   (alternating DVE/Act) and DMA'd out per 128-token row block.
"""

import sys

for _p in ("/opt/trn_rl_repo",):
    if _p not in sys.path:
        sys.path.insert(0, _p)

import numpy as np
import ml_dtypes
from contextlib import ExitStack

import concourse.bass as bass
import concourse.mybir as mybir
import concourse.tile as tile
from concourse import bacc
from concourse.bass_utils import run_bass_kernel_spmd

F32R = mybir.dt.float32r
F32 = mybir.dt.float32
BF16 = mybir.dt.bfloat16
AF = mybir.ActivationFunctionType
BF = ml_dtypes.bfloat16

B, T, D, KV = 4, 1024, 2048, 512
H, HKV, HD = 32, 8, 64
P = 128
NCORES = 8
HPC = H // NCORES          # 4 q heads per core
DOUT = HPC * HD            # 256 q-proj cols per core
NDC = D // P               # 16 contraction chunks
NTC = T // P               # 8 token chunks per batch
ROPE_BASE = 10000.0

_COMPILED = {}


def _chunks_for(kc):
    """Natural S/PV q-chunks for key block kc (bf16: any width is full
    rate). Chunks never straddle the 512 boundary (PSUM half split)."""
    q0 = kc * P
    out = []
    c0 = q0
    while c0 < T:
        c1 = min(T, 512 if c0 < 512 else T)
        out.append((c0, c1))
        c0 = c1
    return q0, out


def _build_nc():
    nc = bacc.Bacc("TRN2", target_bir_lowering=False, debug=False,
                   num_devices=NCORES)

    xT = nc.dram_tensor("xT", [D, B * T], BF16, kind="ExternalInput")
    wq = nc.dram_tensor("wq", [P, NDC, DOUT], BF16, kind="ExternalInput")
    wkv = nc.dram_tensor("wkv", [P, NDC, 2 * HD], BF16, kind="ExternalInput")
    wo = nc.dram_tensor("wo", [P, 2, D], BF16, kind="ExternalInput")
    cc = nc.dram_tensor("cc", [P, T], F32, kind="ExternalInput")
    ss = nc.dram_tensor("ss", [P, T], F32, kind="ExternalInput")
    tri2 = nc.dram_tensor("tri2", [P, 2, P], BF16, kind="ExternalInput")
    o64 = nc.dram_tensor("o64", [64, 2], F32R, kind="ExternalInput")
    o1x64 = nc.dram_tensor("o1x64", [1, 64], F32R, kind="ExternalInput")
    idb = nc.dram_tensor("idb", [64, 64], BF16, kind="ExternalInput")
    out = nc.dram_tensor("out", [B * T, D], BF16, kind="ExternalOutput")

    with tile.TileContext(nc) as tc:
        with ExitStack() as ctx:
            cpool = ctx.enter_context(tc.tile_pool(name="consts", bufs=1))
            wpool = ctx.enter_context(tc.tile_pool(name="weights", bufs=1))
            xpool = ctx.enter_context(tc.tile_pool(name="x", bufs=8))
            spool = ctx.enter_context(tc.tile_pool(name="stage", bufs=4))
            qpool = ctx.enter_context(tc.tile_pool(name="qk", bufs=2))
            vpool = ctx.enter_context(tc.tile_pool(name="v", bufs=2))
            fpool = ctx.enter_context(tc.tile_pool(name="pt", bufs=3))
            npool = ctx.enter_context(tc.tile_pool(name="norm", bufs=4))
            ypool = ctx.enter_context(tc.tile_pool(name="ytn", bufs=2))
            opool = ctx.enter_context(tc.tile_pool(name="ostg", bufs=3))
            psy = ctx.enter_context(tc.tile_pool(name="psy", bufs=2, space="PSUM"))
            pss = ctx.enter_context(tc.tile_pool(name="pss", bufs=4, space="PSUM"))
            psm = ctx.enter_context(tc.tile_pool(name="psm", bufs=2, space="PSUM"))

            # ---- early consts (needed by first projections/rope) ----
            t_wkv = wpool.tile([P, NDC, 2 * HD], BF16)
            nc.sync.dma_start(t_wkv[:], wkv.ap())
            t_wq = wpool.tile([P, NDC, DOUT], BF16)
            t_cc = cpool.tile([P, T], F32)
            t_ss = cpool.tile([P, T], F32)

            xT3 = xT.ap().rearrange("(c p) t -> p c t", p=P)  # [128, 16, 4096]

            def late_consts():
                t_tri2 = cpool.tile([P, 2, P], BF16)
                nc.sync.dma_start(t_tri2[:], tri2.ap())
                t_o64 = cpool.tile([64, 2], F32R)
                nc.sync.dma_start(t_o64[:], o64.ap())
                t_o1x64 = cpool.tile([1, 64], F32R)
                nc.sync.dma_start(t_o1x64[:], o1x64.ap())
                t_idb = cpool.tile([64, 64], BF16)
                nc.sync.dma_start(t_idb[:], idb.ap())
                t_wo = wpool.tile([P, 2, D], BF16)
                nc.sync.dma_start(t_wo[:], wo.ap())
                return t_tri2, t_o64, t_wo, t_o1x64, t_idb

            lc = None

            def rope_half(pj, dst, rows, half, sign_dup):
                """Evict PSUM proj chunk, rope it, write bf16 into dst."""
                c0 = half * 512
                sb = spool.tile([P, 512], F32, tag="qsb")
                nc.vector.tensor_copy(sb[0:rows, :], pj[0:rows, :])
                t1 = spool.tile([P, 512], F32, tag="t1")
                nc.vector.tensor_mul(
                    t1[0:rows, :], sb[0:rows, :], t_cc[0:rows, c0:c0 + 512])
                t2 = spool.tile([P, 512], F32, tag="t2")
                for bp2 in range(0, rows, 64):
                    nc.gpsimd.tensor_mul(
                        t2[bp2:bp2 + 32, :], sb[bp2 + 32:bp2 + 64, :],
                        t_ss[bp2 + 32:bp2 + 64, c0:c0 + 512])
                    nc.gpsimd.tensor_mul(
                        t2[bp2 + 32:bp2 + 64, :], sb[bp2:bp2 + 32, :],
                        t_ss[bp2:bp2 + 32, c0:c0 + 512])
                nc.vector.tensor_add(
                    dst[0:rows, c0:c0 + 512], t1[0:rows, :], t2[0:rows, :])
                if sign_dup:
                    nc.vector.tensor_copy(
                        dst[64:128, c0:c0 + 512], dst[0:64, c0:c0 + 512])

            def proj_rope_stage(b):
                """Load xT for batch b, project Q/K/V, rope, prep vh/kb."""
                nonlocal lc
                tok0 = b * T
                xts = []
                for qtr in range(4):
                    xt = xpool.tile([P, 4, T], BF16, tag="xt",
                                    name=f"xt_{b}_{qtr}")
                    if b == 0:
                        # fine-grained loads so batch-0 projections start
                        # as soon as the first contraction chunk lands
                        for i in range(4):
                            nc.sync.dma_start(
                                xt[:, i, :],
                                xT3[:, qtr * 4 + i, tok0:tok0 + T])
                        if qtr == 0:
                            nc.sync.dma_start(t_wq[:], wq.ap())
                        if qtr == 2:
                            nc.sync.dma_start(t_cc[:], cc.ap())
                            nc.sync.dma_start(t_ss[:], ss.ap())
                    else:
                        nc.sync.dma_start(
                            xt[:], xT3[:, qtr * 4:(qtr + 1) * 4, tok0:tok0 + T])
                    xts.append(xt)
                if b == 0:
                    lc = late_consts()

                def xsrc(dc):
                    return xts[dc // 4][:, dc % 4, :]

                qh = [qpool.tile([P, T], BF16, tag="qh", bufs=4,
                                 name=f"qh_{b}_{i}") for i in range(2)]
                kh = qpool.tile([P, T], BF16, tag="kh", name=f"kh_{b}")
                k2 = qpool.tile([64, T], F32R, tag="k2", name=f"k2_{b}")
                vt = vpool.tile([64, T], BF16, tag="vt", name=f"vt_{b}")

                # interleave kv and q-pair0 chunks so both pj slots
                # stream against arriving xT chunks; then q-pair1.
                def kv_chunk(half):
                    pj = psm.tile([P, 512], F32, tag="pj",
                                  name=f"kvpj_{b}_{half}")
                    for dc in range(NDC):
                        nc.tensor.matmul(
                            pj[:], t_wkv[:, dc, :],
                            xsrc(dc)[:, half * 512:(half + 1) * 512],
                            start=(dc == 0), stop=(dc == NDC - 1))
                    # kv proj out rows 0:64 = K dims, 64:128 = V dims.
                    nc.vector.tensor_copy(vt[:, half * 512:(half + 1) * 512],
                                          pj[64:128, :])
                    rope_half(pj, kh, 64, half, sign_dup=True)

                def q_chunk(pairi, half):
                    pj = psm.tile([P, 512], F32, tag="pj",
                                  name=f"qpj_{b}_{pairi}_{half}")
                    for dc in range(NDC):
                        nc.tensor.matmul(
                            pj[:],
                            t_wq[:, dc, pairi * P:(pairi + 1) * P],
                            xsrc(dc)[:, half * 512:(half + 1) * 512],
                            start=(dc == 0), stop=(dc == NDC - 1))
                    rope_half(pj, qh[pairi], 128, half, sign_dup=False)

                kv_chunk(0)
                q_chunk(0, 0)
                kv_chunk(1)
                q_chunk(0, 1)
                q_chunk(1, 0)
                q_chunk(1, 1)

                # |k|^2 -> per-partition bias  (transposed via PE)
                t_o64, t_idb = lc[1], lc[4]
                nc.scalar.activation(k2[:], kh[0:64, :], AF.Square)
                nsq = psm.tile([P, 512], F32, tag="pj", name=f"nsq_{b}")
                for kc in range(NTC):
                    nc.tensor.matmul(
                        nsq[:, 2 * kc:2 * kc + 2], k2[:, kc * P:(kc + 1) * P],
                        t_o64[:], start=True, stop=True)
                kb = qpool.tile([P, NTC], F32, tag="kb", name=f"kb_{b}")
                nc.vector.tensor_scalar_mul(
                    kb[:],
                    nsq[:, 0:2 * NTC]
                    .rearrange("p (c two) -> p c two", two=2)[:, :, 0],
                    -0.0625)

                # V transposed into [token, hd | 1] layout via PE transpose
                vh = vpool.tile([P, NTC, HD + 1], BF16, tag="vh",
                                name=f"vh_{b}")
                nc.vector.memset(vh[:, :, HD:HD + 1], 1.0)
                for tcn in range(NTC):
                    tp = pss.tile([P, 64], BF16, tag="stp", name=f"tp_{b}_{tcn}")
                    nc.tensor.transpose(
                        tp[:], vt[:, tcn * P:(tcn + 1) * P], t_idb[:])
                    nc.scalar.copy(vh[:, tcn, 0:HD], tp[:])
                return dict(b=b, qh=qh, kh=kh, kb=kb, vh=vh)

            def attn_out_stage(st):
                b, qh, kh, kb, vh = st["b"], st["qh"], st["kh"], st["kb"], st["vh"]
                tok0 = b * T
                t_tri2, t_o64, t_wo, t_o1x64, t_idb = lc
                ytn = [ypool.tile([P, T], BF16, tag="ytn", bufs=4,
                                  name=f"ytn_{b}_{i}") for i in range(2)]
                def normalize_half(yh_half, hq, pairi, bp, tag):
                    """One half of softmax-normalize as soon as its PV
                    contributions are complete; frees the yh slot early."""
                    rsb = npool.tile([1, 512], F32R, tag="rsb")
                    with nc.allow_low_precision(reason="recip row"):
                        nc.vector.reciprocal(rsb[:], yh_half[64:65, :])
                    rbc = npool.tile([64, 512], F32R, tag="rbc")
                    nc.gpsimd.partition_broadcast(rbc[:], rsb[:])
                    nc.vector.tensor_mul(
                        ytn[pairi][bp:bp + 64, hq * 512:(hq + 1) * 512],
                        yh_half[0:64, :], rbc[:])

                for h in range(HPC):
                    pairi, bp = h // 2, 64 * (h % 2)
                    yhA = psy.tile([65, 512], F32, tag="yh",
                                   name=f"yhA_{b}_{h}")
                    yhB = psy.tile([65, 512], F32, tag="yh",
                                   name=f"yhB_{b}_{h}")
                    for kc in range(NTC):
                        q0 = kc * P
                        _, chs = _chunks_for(kc)
                        pt = fpool.tile([P, T], BF16, tag="pt", bufs=6)
                        for (c0, c1) in chs:
                            stp = pss.tile([P, 512], F32, tag="stp")
                            nc.tensor.matmul(
                                stp[:, 0:c1 - c0],
                                kh[bp:bp + 64, kc * P:(kc + 1) * P],
                                qh[pairi][bp:bp + 64, c0:c1],
                                start=True, stop=True,
                                tile_position=(bp, 0))
                            nc.scalar.activation(
                                pt[:, c0:c1], stp[:, 0:c1 - c0], AF.Exp,
                                bias=kb[:, kc:kc + 1], scale=0.125)
                        # causal mask on the diagonal block
                        meng = nc.vector if kc % 2 == 0 else nc.gpsimd
                        meng.tensor_mul(
                            pt[:, q0:q0 + P], pt[:, q0:q0 + P],
                            t_tri2[:, 0, :])
                        for (c0, c1) in chs:
                            half = yhA if c0 < 512 else yhB
                            off = 0 if c0 < 512 else 512
                            nc.tensor.matmul(
                                half[:, c0 - off:c1 - off], vh[:, kc, :],
                                pt[:, c0:c1],
                                start=(kc == 0),
                                stop=(kc == (3 if half is yhA else NTC - 1)),
                                skip_group_check=True)
                        if kc == 3:
                            normalize_half(yhA, 0, pairi, bp, "A")
                    normalize_half(yhB, 1, pairi, bp, "B")

                # ---------- output projection ----------
                for tcn in range(NTC):
                    ostg = opool.tile([P, D], BF16, tag="ostg")
                    for oc in range(4):
                        ops_ = pss.tile([P, 512], F32, tag="stp",
                                        name=f"ops_{b}_{tcn}_{oc}")
                        for hc in range(2):
                            nc.tensor.matmul(
                                ops_[:], ytn[hc][:, tcn * P:(tcn + 1) * P],
                                t_wo[:, hc, oc * 512:(oc + 1) * 512],
                                start=(hc == 0), stop=(hc == 1))
                        if oc % 2 == 1:
                            nc.scalar.copy(
                                ostg[:, oc * 512:(oc + 1) * 512], ops_[:])
                        else:
                            nc.vector.tensor_copy(
                                ostg[:, oc * 512:(oc + 1) * 512], ops_[:])
                    nc.sync.dma_start(
                        out.ap()[tok0 + tcn * P: tok0 + (tcn + 1) * P, :],
                        ostg[:])

            # software pipeline: proj/rope of b+1 issued (higher priority)
            # before attention/outproj of b so PE always has filler work.
            prev = proj_rope_stage(0)
            for b in range(1, B):
                cur = proj_rope_stage(b)
                attn_out_stage(prev)
                prev = cur
            attn_out_stage(prev)

    nc.finalize()
    return nc


def _host_consts():
    inv = 1.0 / (ROPE_BASE ** (np.arange(0, HD, 2, dtype=np.float32) / HD))
    ang = np.arange(T, dtype=np.float32)[:, None] * inv[None, :]  # [T, 32]
    cosr = np.cos(ang).T.astype(np.float32)                        # [32, T]
    sinr = np.sin(ang).T.astype(np.float32)
    cc = np.tile(cosr, (4, 1))                                     # [128, T]
    ss = np.tile(np.concatenate([sinr, -sinr], axis=0), (2, 1))
    tri = np.triu(np.ones((P, P), np.float32))
    tri2 = np.stack([tri, tri], axis=1)                            # [128,2,128]
    return {
        "cc": np.ascontiguousarray(cc),
        "ss": np.ascontiguousarray(ss),
        "tri2": np.ascontiguousarray(tri2.astype(BF)),
        "o64": np.ones((64, 2), np.float32),
        "o1x64": np.ones((1, 64), np.float32),
        "idb": np.eye(64, dtype=np.float32).astype(BF),
    }


def kernel(x, Wq, Wk, Wv, Wo):
    x = np.asarray(x, np.float32)
    Wq = np.asarray(Wq, np.float32)
    Wk = np.asarray(Wk, np.float32)
    Wv = np.asarray(Wv, np.float32)
    Wo = np.asarray(Wo, np.float32)
    b, t, d = x.shape

    key = "nc"
    if key not in _COMPILED:
        _COMPILED[key] = _build_nc()
    nc = _COMPILED[key]

    xTh = np.ascontiguousarray(x.reshape(b * t, d).T.astype(BF))  # [2048, 4096]
    consts = _host_consts()

    in_maps = []
    for c in range(NCORES):
        wq_c = np.ascontiguousarray(
            Wq[:, c * DOUT:(c + 1) * DOUT].reshape(NDC, P, DOUT)
            .transpose(1, 0, 2).astype(BF))
        wkv_np = np.concatenate(
            [Wk[:, c * HD:(c + 1) * HD], Wv[:, c * HD:(c + 1) * HD]], axis=1)
        wkv_c = np.ascontiguousarray(
            wkv_np.reshape(NDC, P, 2 * HD).transpose(1, 0, 2).astype(BF))
        wo_c = np.ascontiguousarray(
            Wo[c * DOUT:(c + 1) * DOUT, :].reshape(2, P, d)
            .transpose(1, 0, 2).astype(BF))
        m = {"xT": xTh, "wq": wq_c, "wkv": wkv_c, "wo": wo_c}
        m.update(consts)
        in_maps.append(m)

    res = run_bass_kernel_spmd(nc, in_maps, list(range(NCORES)))
    acc = res.results[0]["out"].astype(np.float32)
    for c in range(1, NCORES):
        acc = acc + res.results[c]["out"].astype(np.float32)
    return acc.reshape(b, t, d)


if __name__ == "__main__":
    rng = np.random.default_rng(0)
    x = rng.standard_normal((B, T, D), dtype=np.float32)
    Wq = (rng.standard_normal((D, D), dtype=np.float32) * 0.02)
    Wk = (rng.standard_normal((D, KV), dtype=np.float32) * 0.02)
    Wv = (rng.standard_normal((D, KV), dtype=np.float32) * 0.02)
    Wo = (rng.standard_normal((D, D), dtype=np.float32) * 0.02)
    y = kernel(x=x, Wq=Wq, Wk=Wk, Wv=Wv, Wo=Wo)
    print("out", y.shape, y.dtype, np.abs(y).max())
